# revision 8
# baseline (speedup 1.0000x reference)
"""Trainium2 Bass kernel for nn_CEM_86517821216128 (CEM planner with DTW distance).

Strategy: pure data-parallel over the flattened population*batch axis across 8
NeuronCores (32 population candidates x 8 batch per core). The full 2-iteration
CEM loop runs in ONE kernel launch:
  iter1: rollout -> decode-free cost (via projected obs + Cholesky Gram trick)
         -> DTW wavefront DP -> dists
  AllGather(dists [P,B]) -> on-device exact top-K elite weights
  AllReduce(weighted action moments) -> means/stds -> iter2 actions
  iter2: rollout -> cost -> DTW (+ backtrace directions) -> dump outputs
Host does: deterministic jax.random noise precompute, input re-layout, final
argmin/backtrace/reward (tiny), gather of per-core outputs.
"""

import numpy as np

import concourse.bass as bass
import concourse.bacc as bacc
import concourse.mybir as mybir
import concourse.tile as tile
from concourse.masks import make_identity

F32 = mybir.dt.float32
AL = mybir.AluOpType
AF = mybir.ActivationFunctionType
AX = mybir.AxisListType

NCORE = 8
B = 8
PL = 32            # population per core
P = NCORE * PL     # 256
PBL = PL * B       # 256 candidates per core
T = 12
A = 6
HB = 200
SS = 30
NZ = HB + SS       # 230
KE = int(P * 0.1)  # 25 elites
TEMP = 0.5
MOM = 0.1
MIN_STD = 0.05
BIG = 1e30
C9 = 1.0 / (1.0 + 1e-9)


def _ap(t, off, dims):
    return bass.AP(t.tensor, t.offset + off, [t.ap[0]] + dims)


def build_nc():
    nc = bacc.Bacc(num_devices=NCORE)

    # ---- per-core external inputs
    wd0 = nc.dram_tensor("wd0", [128, HB], F32, kind="ExternalInput")
    wd1 = nc.dram_tensor("wd1", [128, HB], F32, kind="ExternalInput")
    wd2 = nc.dram_tensor("wd2", [8, HB], F32, kind="ExternalInput")
    ws0 = nc.dram_tensor("ws0", [128, SS], F32, kind="ExternalInput")
    ws1 = nc.dram_tensor("ws1", [72, SS], F32, kind="ExternalInput")
    lc0 = nc.dram_tensor("lc0", [128, NZ], F32, kind="ExternalInput")
    lc1 = nc.dram_tensor("lc1", [72, NZ], F32, kind="ExternalInput")
    lc2 = nc.dram_tensor("lc2", [30, NZ], F32, kind="ExternalInput")
    ot0 = nc.dram_tensor("ot0", [128, 96], F32, kind="ExternalInput")
    ot1 = nc.dram_tensor("ot1", [72, 96], F32, kind="ExternalInput")
    ot2 = nc.dram_tensor("ot2", [30, 96], F32, kind="ExternalInput")
    o2b_d = nc.dram_tensor("o2b", [1, 96], F32, kind="ExternalInput")
    b0f = nc.dram_tensor("b0f", [HB, PBL], F32, kind="ExternalInput")
    s0f = nc.dram_tensor("s0f", [SS, PBL], F32, kind="ExternalInput")
    act1_d = nc.dram_tensor("act1", [A, T * PBL], F32, kind="ExternalInput")
    noi2_d = nc.dram_tensor("noi2", [A, T * PBL], F32, kind="ExternalInput")
    lsel_d = nc.dram_tensor("lsel", [B, P], F32, kind="ExternalInput")

    # ---- per-core external outputs
    dists2_o = nc.dram_tensor("dists2_o", [PBL], F32, kind="ExternalOutput")
    dirs_o = nc.dram_tensor("dirs_o", [128, 288], F32, kind="ExternalOutput")
    bel_o = nc.dram_tensor("bel_o", [HB, T * PBL], F32, kind="ExternalOutput")
    sta_o = nc.dram_tensor("sta_o", [SS, T * PBL], F32, kind="ExternalOutput")

    with tile.TileContext(nc) as tc:
        with (
            tc.tile_pool(name="cn", bufs=1) as cn,
            tc.tile_pool(name="wk", bufs=1) as wk,
            tc.tile_pool(name="pp", bufs=1, space=bass.MemorySpace.PSUM) as pp,
            tc.tile_pool(name="dr", bufs=1, space="DRAM") as dr,
        ):
            # ---------- constants into SBUF
            wd0s = cn.tile([128, HB], F32); nc.sync.dma_start(wd0s[:], wd0[:])
            wd1s = cn.tile([128, HB], F32); nc.sync.dma_start(wd1s[:], wd1[:])
            wd2s = cn.tile([8, HB], F32); nc.sync.dma_start(wd2s[:], wd2[:])
            ws0s = cn.tile([128, SS], F32); nc.sync.dma_start(ws0s[:], ws0[:])
            ws1s = cn.tile([72, SS], F32); nc.sync.dma_start(ws1s[:], ws1[:])
            lc0s = cn.tile([128, NZ], F32); nc.sync.dma_start(lc0s[:], lc0[:])
            lc1s = cn.tile([72, NZ], F32); nc.sync.dma_start(lc1s[:], lc1[:])
            lc2s = cn.tile([30, NZ], F32); nc.sync.dma_start(lc2s[:], lc2[:])
            ot0s = cn.tile([128, 96], F32); nc.sync.dma_start(ot0s[:], ot0[:])
            ot1s = cn.tile([72, 96], F32); nc.sync.dma_start(ot1s[:], ot1[:])
            ot2s = cn.tile([30, 96], F32); nc.sync.dma_start(ot2s[:], ot2[:])
            o2bs = cn.tile([1, 96], F32); nc.sync.dma_start(o2bs[:], o2b_d[:])
            act1s = cn.tile([A, T * PBL], F32); nc.sync.dma_start(act1s[:], act1_d[:])
            b0fA = cn.tile([128, PBL], F32); nc.sync.dma_start(b0fA[:], b0f[0:128, :])
            b0fB = cn.tile([72, PBL], F32); nc.sync.dma_start(b0fB[:], b0f[128:200, :])
            s0s = cn.tile([SS, PBL], F32); nc.sync.dma_start(s0s[:], s0f[:])
            noi2s = cn.tile([A, T * PBL], F32); nc.sync.dma_start(noi2s[:], noi2_d[:])
            lsels = cn.tile([B, P], F32); nc.sync.dma_start(lsels[:], lsel_d[:])

            ident12 = cn.tile([12, 12], F32)
            make_identity(nc, ident12[:])
            ones128 = cn.tile([128, 1], F32); nc.vector.memset(ones128[:], 1.0)
            ones1_12 = cn.tile([1, 12], F32); nc.vector.memset(ones1_12[:], 1.0)
            ones384 = cn.tile([1, 384], F32); nc.vector.memset(ones384[:], 1.0)
            bigt = cn.tile([B, P], F32); nc.vector.memset(bigt[:], BIG)

            # ---------- persistent work tiles
            x0 = wk.tile([128, PBL], F32)
            x1 = wk.tile([128, PBL], F32)
            x2 = wk.tile([8, PBL], F32)
            nc.vector.memset(x0[:], 0.0)
            zb0 = wk.tile([128, T * PBL], F32)
            zb1 = wk.tile([72, T * PBL], F32)
            zs = wk.tile([30, T * PBL], F32)
            r2sb = wk.tile([1, T * PBL], F32)
            dot_sb = wk.tile([12, 8 * 384], F32)
            cost = wk.tile([128, 288], F32)
            mean_s = wk.tile([A, 96], F32)
            std_s = wk.tile([A, 96], F32)

            # DRAM scratch (tile-tracked)
            d1loc = dr.tile([PBL], F32)
            d1all = dr.tile([NCORE * PBL], F32, addr_space="Shared")
            momloc = dr.tile([A, 200], F32)
            momall = dr.tile([A, 200], F32, addr_space="Shared")
            w6scr = dr.tile([PBL], F32)

            for it in range(2):
                # ============ rollout ============
                nc.vector.tensor_copy(x0[0:30, :], s0s[:])
                nc.vector.tensor_copy(x0[64:128, :], b0fA[0:64, :])
                nc.scalar.activation(x1[0:64, :], b0fA[64:128, :], AF.Copy)
                nc.scalar.activation(x1[64:128, :], b0fB[0:64, :], AF.Copy)
                nc.vector.tensor_copy(x2[0:8, :], b0fB[64:72, :])
                for t in range(T):
                    c0 = t * PBL
                    if it == 0:
                        nc.vector.tensor_copy(x0[32:38, :], act1s[:, c0:c0 + PBL])
                    else:
                        tmp6 = wk.tile([A, PBL], F32, tag="tmp6", bufs=2)
                        mb = _ap(std_s, t * 8, [[1, 8], [0, 32]])
                        nc.vector.tensor_tensor(tmp6[:], noi2s[:, c0:c0 + PBL], mb, AL.mult)
                        mb2 = _ap(mean_s, t * 8, [[1, 8], [0, 32]])
                        nc.vector.tensor_tensor(tmp6[:], tmp6[:], mb2, AL.add)
                        nc.vector.tensor_scalar(x0[32:38, :], tmp6[:], 1.0, -1.0, op0=AL.min, op1=AL.max)

                    pb0 = pp.tile([128, PBL], F32, tag="pA")
                    pb1 = pp.tile([72, PBL], F32, tag="pB")
                    nc.tensor.matmul(pb0[:], wd0s[:, 0:128], x0[:], start=True, stop=False)
                    nc.tensor.matmul(pb0[:], wd1s[:, 0:128], x1[:], start=False, stop=False)
                    nc.tensor.matmul(pb0[:], wd2s[:, 0:128], x2[:], start=False, stop=True)
                    nc.tensor.matmul(pb1[:], wd0s[:, 128:200], x0[:], start=True, stop=False)
                    nc.tensor.matmul(pb1[:], wd1s[:, 128:200], x1[:], start=False, stop=False)
                    nc.tensor.matmul(pb1[:], wd2s[:, 128:200], x2[:], start=False, stop=True)

                    nc.scalar.activation(zb0[:, c0:c0 + PBL], pb0[:], AF.Tanh)
                    nc.scalar.activation(zb1[0:72, c0:c0 + PBL], pb1[:], AF.Tanh)
                    if t < T - 1:
                        nc.scalar.activation(x0[64:128, :], pb0[0:64, :], AF.Tanh)
                        nc.scalar.activation(x1[0:64, :], pb0[64:128, :], AF.Tanh)
                        nc.scalar.activation(x1[64:128, :], pb1[0:64, :], AF.Tanh)
                        nc.scalar.activation(x2[0:8, :], pb1[64:72, :], AF.Tanh)

                    ps = pp.tile([30, PBL], F32, tag="pC")
                    nc.tensor.matmul(ps[:], ws0s[:], zb0[:, c0:c0 + PBL], start=True, stop=False)
                    nc.tensor.matmul(ps[:], ws1s[:], zb1[0:72, c0:c0 + PBL], start=False, stop=True)
                    nc.scalar.activation(zs[:, c0:c0 + PBL], ps[:], AF.Tanh)
                    if t < T - 1:
                        nc.scalar.activation(x0[0:30, :], ps[:], AF.Tanh)

                    if it == 1:
                        nc.sync.dma_start(
                            bass.AP(bel_o, c0, [[T * PBL, 128], [1, PBL]]), zb0[:, c0:c0 + PBL])
                        nc.sync.dma_start(
                            bass.AP(bel_o, 128 * T * PBL + c0, [[T * PBL, 72], [1, PBL]]), zb1[0:72, c0:c0 + PBL])
                        nc.sync.dma_start(
                            bass.AP(sta_o, c0, [[T * PBL, 30], [1, PBL]]), zs[:, c0:c0 + PBL])

                # ============ r2 = |L^T z|^2 ============
                for nt in range(6):
                    s0c = nt * 512
                    v0 = pp.tile([128, 512], F32, tag="pA")
                    v1 = pp.tile([102, 512], F32, tag="pB")
                    for ki, (lcs, zrow) in enumerate([(lc0s, zb0), (lc1s, zb1), (lc2s, zs)]):
                        zap = zrow[:, s0c:s0c + 512] if ki == 0 else (
                            zb1[0:72, s0c:s0c + 512] if ki == 1 else zs[0:30, s0c:s0c + 512])
                        nc.tensor.matmul(v0[:], lcs[:, 0:128], zap, start=(ki == 0), stop=(ki == 2))
                        nc.tensor.matmul(v1[:], lcs[:, 128:230], zap, start=(ki == 0), stop=(ki == 2))
                    sq0 = wk.tile([128, 512], F32, tag="sq0", bufs=2)
                    sq1 = wk.tile([102, 512], F32, tag="sq1", bufs=2)
                    nc.scalar.activation(sq0[:], v0[:], AF.Square)
                    nc.scalar.activation(sq1[:], v1[:], AF.Square)
                    r2p = pp.tile([1, 512], F32, tag="pC")
                    nc.tensor.matmul(r2p[:], ones128[:], sq0[:], start=True, stop=False)
                    nc.tensor.matmul(r2p[:], ones128[0:102, :], sq1[:], start=False, stop=True)
                    nc.scalar.activation(r2sb[:, s0c:s0c + 512], r2p[:], AF.Copy)

                # ============ dot + cost assembly ============
                for b in range(B):
                    dps = pp.tile([12, 384], F32, tag="pD")
                    otb0 = _ap(ot0s, b, [[8, 12]])
                    otb1 = _ap(ot1s, b, [[8, 12]])
                    otb2 = _ap(ot2s, b, [[8, 12]])
                    rz0 = _ap(zb0, b * 32, [[PBL, 12], [1, 32]])
                    rz1 = _ap(zb1, b * 32, [[PBL, 12], [1, 32]])
                    rz2 = _ap(zs, b * 32, [[PBL, 12], [1, 32]])
                    rr2 = _ap(r2sb, b * 32, [[PBL, 12], [1, 32]])
                    ob = _ap(o2bs, b, [[8, 12]])
                    nc.tensor.matmul(dps[:], otb0, rz0, start=True, stop=False)
                    nc.tensor.matmul(dps[:], otb1, rz1, start=False, stop=False)
                    nc.tensor.matmul(dps[:], otb2, rz2, start=False, stop=False)
                    nc.tensor.matmul(dps[:], ones1_12[:], rr2, start=False, stop=False)
                    nc.tensor.matmul(dps[:], ob, ones384[:], start=False, stop=True)
                    nc.scalar.activation(dot_sb[:, b * 384:(b + 1) * 384], dps[:], AF.Copy)

                for b in range(B):
                    h, bq = b // 4, b % 4
                    for c in range(3):
                        trp = pp.tile([128, 12], F32, tag="pE", bufs=2)
                        nc.tensor.transpose(trp[:], dot_sb[:, b * 384 + c * 128:b * 384 + (c + 1) * 128], ident12[:])
                        for js in range(4):
                            j = 4 * c + js
                            dst = bass.AP(cost.tensor, cost.offset + bq * 32 * 288 + h * 144 + j,
                                          [[288, 32], [12, 12]])
                            if js % 2 == 0:
                                nc.scalar.activation(dst, trp[js * 32:(js + 1) * 32, :], AF.Copy)
                            else:
                                nc.vector.tensor_copy(dst, trp[js * 32:(js + 1) * 32, :])

                # ============ DTW ============
                Dp = wk.tile([128, 338], F32, tag="Dp")
                dirs = wk.tile([128, 288], F32, tag="dirs")
                tmin = wk.tile([128, 24], F32, tag="tmin")
                isu = wk.tile([128, 24], F32, tag="isu")
                isl = wk.tile([128, 24], F32, tag="isl")
                nc.vector.memset(Dp[:], BIG)
                for h in range(2):
                    nc.vector.memset(Dp[:, h * 169:h * 169 + 1], 0.0)

                for k in range(2 * T - 1):
                    ilo, ihi = max(0, k - (T - 1)), min(T - 1, k)
                    n = ihi - ilo + 1
                    u = _ap(Dp, 12 * ilo + k + 1, [[169, 2], [12, n]])
                    lft = _ap(Dp, 12 * ilo + k + 13, [[169, 2], [12, n]])
                    g = _ap(Dp, 12 * ilo + k, [[169, 2], [12, n]])
                    dst = _ap(Dp, 12 * ilo + k + 14, [[169, 2], [12, n]])
                    c_ap = _ap(cost, 11 * ilo + k, [[144, 2], [11, n]])
                    tm = _ap(tmin, 0, [[n, 2], [1, n]])
                    nc.vector.tensor_tensor(tm, u, lft, AL.min)
                    nc.vector.tensor_tensor(tm, tm, g, AL.min)
                    nc.vector.tensor_tensor(dst, c_ap, tm, AL.add)
                    if it == 1:
                        d_ap = _ap(dirs, 11 * ilo + k, [[144, 2], [11, n]])
                        iu = _ap(isu, 0, [[n, 2], [1, n]])
                        il = _ap(isl, 0, [[n, 2], [1, n]])
                        nc.vector.tensor_tensor(iu, u, tm, AL.is_le)
                        nc.vector.tensor_tensor(il, lft, tm, AL.is_le)
                        nc.vector.tensor_scalar(il, il, -1.0, 2.0, op0=AL.mult, op1=AL.add)
                        nc.vector.tensor_tensor(iu, il, iu, AL.mult)
                        nc.vector.tensor_tensor(d_ap, il, iu, AL.subtract)

                dsb = wk.tile([128, 2], F32, tag="dsb")
                nc.vector.tensor_copy(dsb[:], _ap(Dp, 168, [[169, 2]]))

                if it == 0:
                    # ---- AllGather dists
                    nc.sync.dma_start(bass.AP(d1loc.tensor, d1loc.offset, [[1, 128], [128, 2]]), dsb[:])
                    nc.gpsimd.collective_compute(
                        "AllGather", AL.bypass,
                        replica_groups=[list(range(NCORE))],
                        ins=[d1loc[:]], outs=[d1all[:]])
                    dall = wk.tile([B, P], F32)
                    nc.sync.dma_start(
                        dall[:],
                        bass.AP(d1all.tensor, d1all.offset, [[32, B], [PBL, NCORE], [1, 32]]))

                    # ---- exact top-K threshold via 25 min-extraction rounds
                    dcur = wk.tile([B, P], F32)
                    nc.vector.tensor_copy(dcur[:], dall[:])
                    m1t = wk.tile([B, 1], F32)
                    tau = wk.tile([B, 1], F32)
                    for k in range(KE):
                        mk = wk.tile([B, 1], F32, tag="mk", bufs=2)
                        nc.vector.tensor_reduce(mk[:], dcur[:], axis=AX.X, op=AL.min)
                        if k == 0:
                            nc.vector.tensor_copy(m1t[:], mk[:])
                        if k == KE - 1:
                            nc.vector.tensor_copy(tau[:], mk[:])
                        else:
                            msk = wk.tile([B, P], F32, tag="msk", bufs=2)
                            nc.vector.scalar_tensor_tensor(msk[:], dcur[:], mk[:], bigt[:], op0=AL.is_equal, op1=AL.mult)
                            nc.vector.tensor_add(dcur[:], dcur[:], msk[:])

                    # ---- w = exp(TEMP*(m1-d)) * (d<=tau) * lsel
                    biasv = wk.tile([B, 1], F32)
                    nc.vector.tensor_scalar_mul(biasv[:], m1t[:], TEMP)
                    e_sb = wk.tile([B, P], F32)
                    nc.scalar.activation(e_sb[:], dall[:], AF.Exp, bias=biasv[:], scale=-TEMP)
                    msk2 = wk.tile([B, P], F32)
                    nc.vector.tensor_scalar(msk2[:], dall[:], tau[:], None, op0=AL.is_le)
                    nc.vector.tensor_mul(e_sb[:], e_sb[:], msk2[:])
                    nc.vector.tensor_mul(e_sb[:], e_sb[:], lsels[:])

                    # ---- local weights [B,32] -> broadcast to [A, PBL]
                    w32 = wk.tile([B, 32], F32)
                    nc.vector.tensor_reduce(w32[:], _ap(e_sb, 0, [[1, 32], [32, 8]]), axis=AX.X, op=AL.add)
                    nc.sync.dma_start(bass.AP(w6scr.tensor, w6scr.offset, [[32, 8], [1, 32]]), w32[:])
                    w6 = wk.tile([A, PBL], F32)
                    nc.sync.dma_start(w6[:], bass.AP(w6scr.tensor, w6scr.offset, [[0, A], [1, PBL]]))

                    # ---- weighted moments of iter-1 actions
                    wa = wk.tile([A, T * PBL], F32)
                    nc.vector.tensor_tensor(wa[:], act1s[:], _ap(w6, 0, [[0, T], [1, PBL]]), AL.mult)
                    wa2 = wk.tile([A, T * PBL], F32)
                    nc.vector.tensor_mul(wa2[:], wa[:], act1s[:])
                    wared = wk.tile([A, 96], F32)
                    nc.vector.tensor_reduce(wared[:], _ap(wa, 0, [[PBL, T], [32, 8], [1, 32]]), axis=AX.X, op=AL.add)
                    wa2red = wk.tile([A, 96], F32)
                    nc.vector.tensor_reduce(wa2red[:], _ap(wa2, 0, [[PBL, T], [32, 8], [1, 32]]), axis=AX.X, op=AL.add)
                    swred = wk.tile([A, 8], F32)
                    nc.vector.tensor_reduce(swred[:], _ap(w6, 0, [[32, 8], [1, 32]]), axis=AX.X, op=AL.add)
                    nc.sync.dma_start(momloc[:, 0:96], wared[:])
                    nc.sync.dma_start(momloc[:, 96:192], wa2red[:])
                    nc.sync.dma_start(momloc[:, 192:200], swred[:])

                    nc.gpsimd.collective_compute(
                        "AllReduce", AL.add,
                        replica_groups=[list(range(NCORE))],
                        ins=[momloc[:]], outs=[momall[:]])

                    moms = wk.tile([A, 200], F32)
                    nc.sync.dma_start(moms[:], momall[:])
                    rw6 = wk.tile([A, 96], F32)
                    nc.vector.reciprocal(rw6[:], _ap(moms, 192, [[0, 12], [1, B]]))

                    m_s = wk.tile([A, 96], F32)
                    nc.vector.tensor_mul(m_s[:], moms[:, 0:96], rw6[:])
                    nc.vector.tensor_scalar_mul(m_s[:], m_s[:], C9)
                    q_s = wk.tile([A, 96], F32)
                    nc.vector.tensor_mul(q_s[:], moms[:, 96:192], rw6[:])
                    nc.vector.tensor_scalar_mul(q_s[:], q_s[:], C9)
                    mm_s = wk.tile([A, 96], F32)
                    nc.vector.tensor_mul(mm_s[:], m_s[:], m_s[:])
                    nc.vector.scalar_tensor_tensor(std_s[:], mm_s[:], -(2.0 - C9), q_s[:], op0=AL.mult, op1=AL.add)
                    nc.vector.tensor_scalar_max(std_s[:], std_s[:], 0.0)
                    nc.scalar.activation(std_s[:], std_s[:], AF.Sqrt)
                    nc.vector.tensor_scalar(std_s[:], std_s[:], MIN_STD, 1.0, op0=AL.max, op1=AL.min)
                    nc.vector.tensor_scalar_mul(mean_s[:], m_s[:], 1.0 - MOM)
                else:
                    nc.sync.dma_start(bass.AP(dists2_o, 0, [[1, 128], [128, 2]]), dsb[:])
                    nc.sync.dma_start(dirs_o[:], dirs[:])

    nc.finalize()
    return nc


# ============================================================ host side

_NOISE = {}


def _get_noise():
    if "n1" not in _NOISE:
        import jax
        with jax.default_device(jax.devices("cpu")[0]):
            key = jax.random.key(42)
            key, s1 = jax.random.split(key)
            n1 = jax.random.normal(s1, (T, P, B, A), "float32")
            key, s2 = jax.random.split(key)
            n2 = jax.random.normal(s2, (T, P, B, A), "float32")
            _NOISE["n1"] = np.asarray(n1)
            _NOISE["n2"] = np.asarray(n2)
    return _NOISE["n1"], _NOISE["n2"]


def _act_layout(arr_c):
    # arr_c [T, PL, B, A] -> [A, T*PBL] with col = t*256 + b*32 + pl
    return np.ascontiguousarray(arr_c.transpose(3, 0, 2, 1).reshape(A, T * PBL))


def _prep_inputs(initial_beliefs, initial_states, observations, W_dyn, W_state, W_obs):
    f = np.float32
    obs = np.asarray(observations, f).reshape(T, B, -1)
    W_dyn = np.asarray(W_dyn, f)
    W_state = np.asarray(W_state, f)
    W_obs = np.asarray(W_obs, f)
    b0 = np.asarray(initial_beliefs, f)
    s0 = np.asarray(initial_states, f)

    o2 = (obs * obs).sum(-1)                       # [T, B]
    Ot = np.einsum("ibd,ed->ibe", obs, W_obs)      # [T, B, NZ]
    ot2 = np.ascontiguousarray((-2.0 * Ot).transpose(2, 0, 1).reshape(NZ, 96)).astype(f)
    Mg = (W_obs @ W_obs.T).astype(np.float64)
    L = np.linalg.cholesky(Mg).astype(f)           # [NZ, NZ]

    wp = np.zeros((264, HB), f)
    wp[0:30] = W_dyn[0:30]        # s rows
    wp[32:38] = W_dyn[30:36]      # a rows
    wp[64:264] = W_dyn[36:236]    # b rows

    shared = {
        "wd0": wp[0:128], "wd1": wp[128:256], "wd2": wp[256:264],
        "ws0": W_state[0:128], "ws1": W_state[128:200],
        "lc0": L[0:128], "lc1": L[128:200], "lc2": L[200:230],
        "ot0": ot2[0:128], "ot1": ot2[128:200], "ot2": ot2[200:230],
        "o2b": np.ascontiguousarray(o2.reshape(1, 96)),
        "b0f": np.ascontiguousarray(np.repeat(b0.T[:, :, None], PL, axis=2).reshape(HB, PBL)),
        "s0f": np.ascontiguousarray(np.repeat(s0.T[:, :, None], PL, axis=2).reshape(SS, PBL)),
    }
    shared = {k: np.ascontiguousarray(v, f) for k, v in shared.items()}

    n1, n2 = _get_noise()
    a1 = np.clip(n1, -1.0, 1.0)
    in_maps = []
    for c in range(NCORE):
        lsel = np.zeros((B, P), f)
        lsel[:, c * PL:(c + 1) * PL] = 1.0
        m = dict(shared)
        m["act1"] = _act_layout(a1[:, c * PL:(c + 1) * PL])
        m["noi2"] = _act_layout(n2[:, c * PL:(c + 1) * PL])
        m["lsel"] = lsel
        in_maps.append(m)
    return in_maps


def _calc_reward_np(routes, rewards):
    # routes [T, T, B] int; rewards [T, B]
    L1, L2, Bb = routes.shape
    out = np.zeros((L1, Bb), np.float32)
    for bb in range(Bb):
        coefs = np.zeros((L1, L2), np.float32)
        i, j = L1 - 1, L2 - 1
        for _ in range(L1 + L2 - 1):
            if i >= 0 and j >= 0:
                coefs[i, j] = 1.0
                r = routes[i, j, bb]
                i -= 1 if r in (0, 2) else 0
                j -= 1 if r in (1, 2) else 0
        coefs = coefs / coefs.sum(axis=0, keepdims=True)
        out[:, bb] = (coefs * rewards[:, bb][None, :]).sum(axis=1)
    return out


def _post(results, rewards):
    rewards = np.asarray(rewards, np.float32)
    dists = np.empty((P, B), np.float32)
    for c in range(NCORE):
        d = np.asarray(results[c]["dists2_o"]).reshape(B, PL)  # [b, pl]
        dists[c * PL:(c + 1) * PL, :] = d.T
    best_i = np.argmin(dists, axis=0)  # [B]

    best_beliefs = np.empty((T, B, HB), np.float32)
    best_states = np.empty((T, B, SS), np.float32)
    routes = np.empty((T, T, B), np.int32)
    for b in range(B):
        p = int(best_i[b])
        c, pl = p // PL, p % PL
        pb = b * PL + pl
        h, r = pb // 128, pb % 128
        best_beliefs[:, b, :] = np.asarray(results[c]["bel_o"])[:, pb::PBL].T
        best_states[:, b, :] = np.asarray(results[c]["sta_o"])[:, pb::PBL].T
        routes[:, :, b] = np.asarray(results[c]["dirs_o"])[r, h * 144:(h + 1) * 144].reshape(T, T).astype(np.int32)
    trans_rewards = _calc_reward_np(routes, rewards)
    return best_beliefs, best_states, trans_rewards


_NC_CACHE = {}


def _get_nc():
    if "nc" not in _NC_CACHE:
        _NC_CACHE["nc"] = build_nc()
    return _NC_CACHE["nc"]


def kernel(initial_beliefs, initial_states, observations, rewards, W_dyn, W_state, W_obs,
           trajectory_length=None, _runner=None):
    in_maps = _prep_inputs(initial_beliefs, initial_states, observations, W_dyn, W_state, W_obs)
    nc = _get_nc()
    if _runner is not None:
        results = _runner(nc, in_maps)
    else:
        from concourse.bass_utils import run_bass_kernel_spmd
        results = run_bass_kernel_spmd(nc, in_maps, list(range(NCORE))).results
    return _post(results, rewards)
